# revision 47
# baseline (speedup 1.0000x reference)
"""Tensor-parallel causal attention kernel for TRN2 (Bass/Tile), v2.

Sharding: 16 heads / 8 cores = 2 heads per core. Each core computes
q,k,v projections for its heads, RoPE, causal attention, and a partial
output projection (row-shard of wo). Host sums the 8 partial outputs.

v2 vs v1: all matmul operands bf16 (PE rows halve nothing but DMA/SBUF
shrink and DVE gets 2-4x); v computed directly in natural layout via
lhsT=x-tile (no PE transposes); softmax denominators via DVE-accumulated
E_acc + one PE ones-matmul per (head,qt) instead of per-chunk matmuls;
1/s broadcast on the Pool engine (partition_broadcast) instead of a PE
outer product; out DMAs (bf16) on the Pool queue, xt loads on SP; and
the per-qt finalize + output projection are software-pipelined into the
next qt's score stream so PE never idles on the DVE/Pool chain.

Layouts (per core):
  xt  [DIM, B*S]   x transposed, bf16 (model dim on partitions)
  wq/wk/wv [DIM, 256] bf16 column slice for this core's 2 heads
  wo  [256, DIM]   bf16 row slice
  cc  [128, S]     [cos.T; cos.T] bf16
  ss  [128, S]     [-sin.T; sin.T] bf16
  maskc [128, MW]  composite causal mask, bf16
  out [B*S, DIM]   partial output (bf16; host sums in fp32)
"""

from contextlib import ExitStack

import numpy as np

import concourse.bass as bass
import concourse.mybir as mybir
import concourse.tile as tile
from concourse import bacc

F32R = mybir.dt.float32r
F32 = mybir.dt.float32
BF16 = mybir.dt.bfloat16
AF = mybir.ActivationFunctionType


def build_nc(B=4, S=2048, DIM=2048, HPC=2, n_cores=8,
             xt_bufs=34, qraw_bufs=4, rot_bufs=4, qfq_bufs=16, kf_bufs=2,
             vn_bufs=32, exp_bufs=8, eacc_bufs=4, rs_bufs=4, rsf_bufs=4,
             ot_bufs=4, op_bufs=12, psum_bufs=8, look=2, drain_lo=6, reps=1):
    P = 128          # partitions
    HD = 128         # head dim
    QT = 512         # query/token tile (moving free dim)
    KC = DIM // P    # contraction chunks for projections
    SC = S // P      # seq 128-chunks per batch
    NQT = S // QT    # q tiles per (b, h)
    JD = QT // P     # 128-sub-blocks per q tile
    MDT = DIM // QT  # model-dim tiles for outproj
    DHC = HPC * HD   # per-core qkv width
    NT = B * S
    MW = (JD - 1) * P + QT  # composite causal mask width
    scale = 1.0 / float(np.sqrt(HD))

    nc = bacc.Bacc("TRN2", target_bir_lowering=False, debug=False,
                   num_devices=n_cores)
    xt = nc.dram_tensor("xt", [DIM, NT], BF16, kind="ExternalInput").ap()
    maskd = nc.dram_tensor("maskc", [P, MW], BF16, kind="ExternalInput").ap()
    wq = nc.dram_tensor("wq", [DIM, DHC], BF16, kind="ExternalInput").ap()
    wk = nc.dram_tensor("wk", [DIM, DHC], BF16, kind="ExternalInput").ap()
    wv = nc.dram_tensor("wv", [DIM, DHC], BF16, kind="ExternalInput").ap()
    wo = nc.dram_tensor("wo", [DHC, DIM], BF16, kind="ExternalInput").ap()
    cc = nc.dram_tensor("cc", [HD, S], BF16, kind="ExternalInput").ap()
    ss = nc.dram_tensor("ss", [HD, S], BF16, kind="ExternalInput").ap()
    onesb = nc.dram_tensor("onesb", [P, 1], BF16, kind="ExternalInput").ap()
    out = nc.dram_tensor("out", [NT, DIM], BF16, kind="ExternalOutput").ap()

    with ExitStack() as ctx:
        tc = ctx.enter_context(tile.TileContext(nc))
        wpool = ctx.enter_context(tc.tile_pool(name="weights", bufs=1))
        xpool = ctx.enter_context(tc.tile_pool(name="xtp", bufs=xt_bufs))
        qrawp = ctx.enter_context(tc.tile_pool(name="qraw", bufs=qraw_bufs))
        rotp = ctx.enter_context(tc.tile_pool(name="rot", bufs=rot_bufs))
        qfp = ctx.enter_context(tc.tile_pool(name="qfp", bufs=qfq_bufs))
        kfp = ctx.enter_context(tc.tile_pool(name="kfp", bufs=kf_bufs))
        vnp = ctx.enter_context(tc.tile_pool(name="vn", bufs=vn_bufs))
        expp = ctx.enter_context(tc.tile_pool(name="expp", bufs=exp_bufs))
        eaccp = ctx.enter_context(tc.tile_pool(name="eacc", bufs=eacc_bufs))
        rsp = ctx.enter_context(tc.tile_pool(name="rs", bufs=rs_bufs))
        rsfp = ctx.enter_context(tc.tile_pool(name="rsf", bufs=rsf_bufs))
        otp = ctx.enter_context(tc.tile_pool(name="ot", bufs=ot_bufs))
        opp = ctx.enter_context(tc.tile_pool(name="op", bufs=op_bufs))
        psum = ctx.enter_context(tc.tile_pool(name="ps", bufs=psum_bufs,
                                              space="PSUM"))

        # ---- persistent constants ----
        wq_t = [wpool.tile([P, DHC], BF16, tag=f"wq{kc}", name=f"wq{kc}")
                for kc in range(KC)]
        wk_t = [wpool.tile([P, DHC], BF16, tag=f"wk{kc}", name=f"wk{kc}")
                for kc in range(KC)]
        wv_t = [wpool.tile([P, DHC], BF16, tag=f"wv{kc}", name=f"wv{kc}")
                for kc in range(KC)]
        for kc in range(KC):
            nc.gpsimd.dma_start(wq_t[kc][:], wq[kc * P:(kc + 1) * P, :])
            nc.gpsimd.dma_start(wk_t[kc][:], wk[kc * P:(kc + 1) * P, :])
            nc.gpsimd.dma_start(wv_t[kc][:], wv[kc * P:(kc + 1) * P, :])
        wo_t = [wpool.tile([P, DIM], BF16, tag=f"wo{h}", name=f"wo{h}")
                for h in range(HPC)]
        for h in range(HPC):
            nc.gpsimd.dma_start(wo_t[h][:], wo[h * HD:(h + 1) * HD, :])
        cc_t = wpool.tile([HD, S], BF16, tag="cc")
        ss_t = wpool.tile([HD, S], BF16, tag="ss")
        nc.gpsimd.dma_start(cc_t[:], cc[:, :])
        nc.gpsimd.dma_start(ss_t[:], ss[:, :])
        maskc = wpool.tile([P, MW], BF16, tag="maskc")
        nc.gpsimd.dma_start(maskc[:], maskd[:, :])
        ones_col = wpool.tile([P, 1], BF16, tag="ones_col")
        nc.gpsimd.dma_start(ones_col[:], onesb[:, :])

        def mask_j(j):
            off = (JD - 1 - j) * P
            return maskc[:, off:off + QT]

        # xt tile prefetch bookkeeping: xts[(b, t, kc)] -> tile
        xts = {}

        def emit_xt_dmas(b):
            tok0 = b * S
            for t in range(NQT):
                for kc in range(KC):
                    xtile = xpool.tile([P, QT], BF16, tag="xt", name="xt")
                    nc.sync.dma_start(
                        xtile[:],
                        xt[kc * P:(kc + 1) * P,
                           tok0 + t * QT:tok0 + (t + 1) * QT])
                    xts[(b, t, kc)] = xtile

        # pending output-projection units (closures), drained to keep PE busy
        pending = []
        fin_pending = []

        def drain(k=None):
            n = len(pending) if k is None else min(k, len(pending))
            for _ in range(n):
                pending.pop(0)()

        def drain_fin(k=1):
            n = len(fin_pending) if k is None else min(k, len(fin_pending))
            for _ in range(n):
                fin_pending.pop(0)()

        def emit_rope(ps_t, dest, tsl):
            """RoPE from psum ps_t -> bf16 dest ([:, tsl] of cc/ss)."""
            raw = qrawp.tile([P, QT], BF16, tag="qraw", name="qraw")
            nc.scalar.copy(raw[:], ps_t[:])
            rot = rotp.tile([P, QT], BF16, tag="rot", name="rot")
            nc.scalar.copy(rot[0:HD // 2, :], raw[HD // 2:HD, :])
            nc.scalar.copy(rot[HD // 2:HD, :], raw[0:HD // 2, :])
            nc.vector.tensor_mul(rot[:], rot[:], ss_t[:, tsl])
            nc.vector.tensor_mul(dest, raw[:], cc_t[:, tsl])
            nc.vector.tensor_add(dest, dest, rot[:])

        for rep in range(reps):
          for b in range(B):
            tok0 = b * S
            if rep == 0 and b == 0:
                emit_xt_dmas(0)
            # ---------- phase A: QKV projections + RoPE ----------
            qf = [[None] * NQT for _ in range(HPC)]
            kf = [kfp.tile([P, S], BF16, tag=f"kf{h}", name=f"kf{h}")
                  for h in range(HPC)]
            vn = [vnp.tile([P, DHC], BF16, tag="vn", name="vn")
                  for _ in range(SC)]
            for t in range(NQT):
                tsl = slice(t * QT, (t + 1) * QT)
                qps = [psum.tile([P, QT], F32, tag="ps", name="ps")
                       for _ in range(HPC)]
                kps = [psum.tile([P, QT], F32, tag="ps", name="ps")
                       for _ in range(HPC)]
                for kc in range(KC):
                    xtile = xts[(b, t, kc)]
                    st = dict(start=(kc == 0), stop=(kc == KC - 1))
                    for h in range(HPC):
                        hsl = slice(h * HD, (h + 1) * HD)
                        nc.tensor.matmul(qps[h][:], wq_t[kc][:, hsl],
                                         xtile[:], **st)
                        nc.tensor.matmul(kps[h][:], wk_t[kc][:, hsl],
                                         xtile[:], **st)
                    if t == 0 and kc in (1, 3):
                        drain_fin()  # prev batch qt=3 finalize stages
                # pending outproj from the previous batch's last qt
                if t == 0:
                    drain()
                # RoPE (ACT+DVE; overlaps the vnat matmuls below on PE)
                for h in range(HPC):
                    qf[h][t] = qfp.tile([P, QT], BF16, tag=f"qf{h}",
                                        name=f"qf{h}")
                    emit_rope(qps[h], qf[h][t][:], tsl)
                    emit_rope(kps[h], kf[h][:, tsl], tsl)
                # v in natural layout: vn[tok128, dv] = sum_kc xt_c.T @ wv_c
                for tcl in range(JD):
                    vp = psum.tile([P, DHC], F32, tag="ps", name="ps")
                    csl = slice(tcl * P, (tcl + 1) * P)
                    for kc in range(KC):
                        nc.tensor.matmul(vp[:], xts[(b, t, kc)][:, csl],
                                         wv_t[kc][:], start=(kc == 0),
                                         stop=(kc == KC - 1))
                    nc.scalar.copy(vn[t * JD + tcl][:], vp[:])

            # ---------- phase B: attention + pipelined outproj ----------
            if b + 1 < B:
                emit_xt_dmas(b + 1)
            elif rep + 1 < reps:
                emit_xt_dmas(0)
            for qt in range(NQT):
                n_kc = JD * (qt + 1)  # causal: key chunks 0..n_kc-1
                avs = [psum.tile([P, QT], F32, tag="ps", name="ps")
                       for _ in range(HPC)]
                eacc = [eaccp.tile([P, QT], BF16, tag="eacc", name="eacc")
                        for _ in range(HPC)]
                ess = [[None] * n_kc for _ in range(HPC)]

                def emit_sc(h, i, qt=qt, ess=ess, eacc=eacc):
                    j = i - JD * qt
                    # causal narrowing: for diagonal chunk j, query columns
                    # < j*P are fully masked -- skip them in sc/exp/av
                    q0 = j * P if 0 < j < JD else 0
                    sc = psum.tile([P, QT], F32, tag="ps", name="ps")
                    nc.tensor.matmul(sc[:, q0:], kf[h][:, i * P:(i + 1) * P],
                                     qf[h][qt][:, q0:], start=True, stop=True)
                    e = expp.tile([P, QT], BF16, tag="exp", name="exp")
                    nc.scalar.activation(e[:, q0:], sc[:, q0:], AF.Exp,
                                         scale=scale)
                    if 0 <= j < JD:
                        tri = maskc[:, (JD - 1) * P:JD * P]
                        nc.vector.tensor_mul(e[:, q0:q0 + P],
                                             e[:, q0:q0 + P], tri)
                    with nc.allow_low_precision(reason="bf16 denom accum"):
                        if i == 0:
                            nc.vector.tensor_copy(eacc[h][:], e[:])
                        else:
                            nc.vector.tensor_add(eacc[h][:, q0:],
                                                 eacc[h][:, q0:], e[:, q0:])
                    ess[h][i] = (e, q0)

                def emit_av(h, i, n_kc=n_kc, avs=avs, ess=ess):
                    e, q0 = ess[h][i]
                    hsl = slice(h * HD, (h + 1) * HD)
                    nc.tensor.matmul(avs[h][:, q0:], vn[i][:, hsl],
                                     e[:, q0:], start=(i == 0),
                                     stop=(i == n_kc - 1),
                                     skip_group_check=True)
                    ess[h][i] = None

                dstride = max(1, (n_kc - drain_lo) // 4)
                dpos = {drain_lo + k * dstride for k in range(4)}
                for i in range(n_kc):
                    for h in range(HPC):
                        emit_sc(h, i)
                    if i == 1 or i == 3:
                        drain_fin()
                    if i >= look:
                        for h in range(HPC):
                            emit_av(h, i - look)
                    if i in dpos:
                        drain(1)
                for i in range(max(0, n_kc - look), n_kc):
                    for h in range(HPC):
                        emit_av(h, i)

                # F1 (deferred): denominators, reciprocal, broadcast, norm
                ots = []

                def fin1(avs=avs, eacc=eacc, ots=ots):
                    for h in range(HPC):
                        smp = psum.tile([1, QT], F32, tag="ps", name="ps")
                        nc.tensor.matmul(smp[:], ones_col[:], eacc[h][:],
                                         start=True, stop=True)
                        rs = rsp.tile([1, QT], F32R, tag="rs", name="rs")
                        with nc.allow_low_precision(reason="f32r width"):
                            nc.vector.reciprocal(rs[:], smp[:])
                        rsf = rsfp.tile([P, QT], F32R, tag="rsf", name="rsf")
                        nc.gpsimd.partition_broadcast(rsf[:], rs[:])
                        ot = otp.tile([P, QT], BF16, tag="ot", name="ot")
                        nc.vector.tensor_mul(ot[:], avs[h][:], rsf[:])
                        ots.append(ot)

                fin_pending.append(fin1)

                # F2: output projection, deferred via pending units
                def make_op_unit(tcl, ots=ots, qt=qt, tok0=tok0):
                    def unit():
                        csl = slice(tcl * P, (tcl + 1) * P)
                        r0 = tok0 + qt * QT + tcl * P
                        for mdt in range(MDT):
                            ps_t = psum.tile([P, QT], F32, tag="ps", name="ps")
                            mdsl = slice(mdt * QT, (mdt + 1) * QT)
                            for h in range(HPC):
                                nc.tensor.matmul(ps_t[:], ots[h][:, csl],
                                                 wo_t[h][:, mdsl],
                                                 start=(h == 0),
                                                 stop=(h == HPC - 1))
                            o = opp.tile([P, QT], BF16, tag="op", name="op")
                            if (tcl * MDT + mdt) % 8 < 3:
                                nc.scalar.copy(o[:], ps_t[:])
                            else:
                                nc.vector.tensor_copy(o[:], ps_t[:])
                            if (tcl * MDT + mdt) % 2 == 0:
                                nc.gpsimd.dma_start(
                                    out[r0:r0 + P, mdsl], o[:])
                            else:
                                nc.sync.dma_start(
                                    out[r0:r0 + P, mdsl], o[:])
                    return unit

                for tcl in range(JD):
                    pending.append(make_op_unit(tcl))
                if rep == reps - 1 and b == B - 1 and qt == NQT - 1:
                    drain_fin(None)
                    drain()
    return nc


def prep_shared(x, cos, sin, QT=512, P=128):
    """Host-side layout prep (transpose/concat/cast only, no FLOPs on x)."""
    import ml_dtypes
    bf16 = ml_dtypes.bfloat16
    B, S, DIM = x.shape
    JD = QT // P
    MW = (JD - 1) * P + QT
    g = np.arange(MW)[None, :]
    p = np.arange(P)[:, None]
    return dict(
        xt=np.ascontiguousarray(x.reshape(B * S, DIM).T).astype(bf16),
        cc=np.ascontiguousarray(
            np.concatenate([cos.T, cos.T], axis=0)).astype(bf16),
        ss=np.ascontiguousarray(
            np.concatenate([-sin.T, sin.T], axis=0)).astype(bf16),
        maskc=(g - (JD - 1) * P - p >= 0).astype(bf16),
        onesb=np.ones((P, 1), dtype=bf16),
    )


def shard_weights(wq, wk, wv, wo, core, n_cores=8, head_dim=128):
    import ml_dtypes
    bf16 = ml_dtypes.bfloat16
    n_heads = wq.shape[1] // head_dim
    hpc = n_heads // n_cores
    dhc = hpc * head_dim
    c0, c1 = core * dhc, (core + 1) * dhc
    return dict(
        wq=np.ascontiguousarray(wq[:, c0:c1]).astype(bf16),
        wk=np.ascontiguousarray(wk[:, c0:c1]).astype(bf16),
        wv=np.ascontiguousarray(wv[:, c0:c1]).astype(bf16),
        wo=np.ascontiguousarray(wo[c0:c1, :]).astype(bf16),
    )


# ---------------------------------------------------------------------------
# Self-contained entry point: kernel(**inputs) -> np.ndarray
# ---------------------------------------------------------------------------
import jax
from jax.sharding import Mesh, PartitionSpec
from jax.experimental.shard_map import shard_map

import concourse.bass2jax as bass2jax

N_CORES = 8
_CACHE = {}


def _get_runner():
    if "runner" in _CACHE:
        return _CACHE["runner"]
    nc = build_nc()
    nc.compile()
    bass2jax.install_neuronx_cc_hook()
    partition_name = (nc.partition_id_tensor.name
                      if nc.partition_id_tensor else None)
    in_names, out_names, out_avals, zero_outs = [], [], [], []
    for alloc in nc.m.functions[0].allocations:
        if not isinstance(alloc, mybir.MemoryLocationSet):
            continue
        name = alloc.memorylocations[0].name
        if alloc.kind == "ExternalInput":
            if name != partition_name:
                in_names.append(name)
        elif alloc.kind == "ExternalOutput":
            shape = tuple(alloc.tensor_shape)
            dtype = mybir.dt.np(alloc.dtype)
            out_names.append(name)
            out_avals.append(jax.core.ShapedArray(shape, dtype))
            zero_outs.append(np.zeros(shape, dtype))
    all_in_names = in_names + out_names
    if partition_name is not None:
        all_in_names = all_in_names + [partition_name]

    def _body(*args):
        operands = list(args)
        if partition_name is not None:
            operands.append(bass2jax.partition_id_tensor())
        outs = bass2jax._bass_exec_p.bind(
            *operands,
            out_avals=tuple(out_avals),
            in_names=tuple(all_in_names),
            out_names=tuple(out_names),
            lowering_input_output_aliases=(),
            sim_require_finite=True,
            sim_require_nnan=True,
            nc=nc,
        )
        return tuple(outs)

    devices = jax.devices()[:N_CORES]
    mesh = Mesh(np.asarray(devices), ("core",))
    n_in = len(in_names) + len(out_names)
    sharded = jax.jit(
        shard_map(_body, mesh=mesh,
                  in_specs=(PartitionSpec("core"),) * n_in,
                  out_specs=(PartitionSpec("core"),) * len(out_names),
                  check_rep=False),
        keep_unused=True,
        donate_argnums=tuple(range(len(in_names), n_in)))
    sharding = jax.sharding.NamedSharding(mesh, PartitionSpec("core"))
    _CACHE["runner"] = (sharded, in_names, out_names, out_avals, zero_outs,
                        sharding)
    return _CACHE["runner"]


def _device_inputs(x, cos, sin, wq, wk, wv, wo):
    shared = prep_shared(np.asarray(x, dtype=np.float32),
                         np.asarray(cos, dtype=np.float32),
                         np.asarray(sin, dtype=np.float32))
    in_maps = []
    for c in range(N_CORES):
        m = dict(shared)
        m.update(shard_weights(np.asarray(wq, dtype=np.float32),
                               np.asarray(wk, dtype=np.float32),
                               np.asarray(wv, dtype=np.float32),
                               np.asarray(wo, dtype=np.float32), c,
                               n_cores=N_CORES))
        in_maps.append(m)
    sharded, in_names, out_names, out_avals, zero_outs, sharding = \
        _get_runner()
    concat_in = [np.concatenate([np.asarray(in_maps[c][n])
                                 for c in range(N_CORES)], axis=0)
                 for n in in_names]
    concat_zero = [np.zeros((N_CORES * z.shape[0], *z.shape[1:]), z.dtype)
                   for z in zero_outs]
    dev_in = [jax.device_put(a, sharding) for a in concat_in + concat_zero]
    for a in dev_in:
        a.block_until_ready()
    return dev_in


def _gather(outs, B, S, DIM):
    full = np.asarray(outs[0]).reshape(N_CORES, B * S, DIM)
    return full.astype(np.float32).sum(axis=0).reshape(B, S, DIM)


def kernel(x, cos, sin, wq, wk, wv, wo):
    """Full inputs in, full output out; work sharded over 8 NeuronCores."""
    B, S, DIM = x.shape
    dev_in = _device_inputs(x, cos, sin, wq, wk, wv, wo)
    sharded = _get_runner()[0]
    outs = sharded(*dev_in)
    jax.block_until_ready(outs)
    return _gather(outs, B, S, DIM)


def measure_hw_time(x, cos, sin, wq, wk, wv, wo, k_lo=5, k_hi=105, trials=3):
    """Marginal per-call time of pipelined executions (min slope).

    Output buffers are donated, so each call's returned output feeds the
    next call's out-init operand (in-place aliasing on device).
    """
    import time as _time
    dev_in = _device_inputs(x, cos, sin, wq, wk, wv, wo)
    sharded, in_names = _get_runner()[:2]
    n_pure = len(in_names)
    pure_in = dev_in[:n_pure]
    state = list(dev_in[n_pure:])
    rs = sharded(*pure_in, *state)
    jax.block_until_ready(rs)
    state = list(rs)

    def timed(k, state):
        t0 = _time.time()
        for _ in range(k):
            state = list(sharded(*pure_in, *state))
        jax.block_until_ready(state)
        return _time.time() - t0, state

    slopes = []
    for _ in range(trials):
        t_lo, state = timed(k_lo, state)
        t_hi, state = timed(k_hi, state)
        slopes.append((t_hi - t_lo) / (k_hi - k_lo))
    return min(slopes)


# revision 48
# speedup vs baseline: 1.0410x; 1.0410x over previous
"""Tensor-parallel causal attention kernel for TRN2 (Bass/Tile), v2.

Sharding: 16 heads / 8 cores = 2 heads per core. Each core computes
q,k,v projections for its heads, RoPE, causal attention, and a partial
output projection (row-shard of wo). Host sums the 8 partial outputs.

v2 vs v1: all matmul operands bf16 (PE rows halve nothing but DMA/SBUF
shrink and DVE gets 2-4x); v computed directly in natural layout via
lhsT=x-tile (no PE transposes); softmax denominators via DVE-accumulated
E_acc + one PE ones-matmul per (head,qt) instead of per-chunk matmuls;
1/s broadcast on the Pool engine (partition_broadcast) instead of a PE
outer product; out DMAs (bf16) on the Pool queue, xt loads on SP; and
the per-qt finalize + output projection are software-pipelined into the
next qt's score stream so PE never idles on the DVE/Pool chain.

Layouts (per core):
  xt  [DIM, B*S]   x transposed, bf16 (model dim on partitions)
  wq/wk/wv [DIM, 256] bf16 column slice for this core's 2 heads
  wo  [256, DIM]   bf16 row slice
  cc  [128, S]     [cos.T; cos.T] bf16
  ss  [128, S]     [-sin.T; sin.T] bf16
  maskc [128, MW]  composite causal mask, bf16
  out [B*S, DIM]   partial output (bf16; host sums in fp32)
"""

from contextlib import ExitStack

import numpy as np

import concourse.bass as bass
import concourse.mybir as mybir
import concourse.tile as tile
from concourse import bacc

F32R = mybir.dt.float32r
F32 = mybir.dt.float32
BF16 = mybir.dt.bfloat16
AF = mybir.ActivationFunctionType


def build_nc(B=4, S=2048, DIM=2048, HPC=2, n_cores=8,
             xt_bufs=34, qraw_bufs=4, rot_bufs=4, qfq_bufs=16, kf_bufs=2,
             vn_bufs=32, exp_bufs=8, eacc_bufs=4, rs_bufs=4, rsf_bufs=4,
             ot_bufs=4, op_bufs=12, psum_bufs=8, look=2, drain_lo=4, reps=1):
    P = 128          # partitions
    HD = 128         # head dim
    QT = 512         # query/token tile (moving free dim)
    KC = DIM // P    # contraction chunks for projections
    SC = S // P      # seq 128-chunks per batch
    NQT = S // QT    # q tiles per (b, h)
    JD = QT // P     # 128-sub-blocks per q tile
    MDT = DIM // QT  # model-dim tiles for outproj
    DHC = HPC * HD   # per-core qkv width
    NT = B * S
    MW = (JD - 1) * P + QT  # composite causal mask width
    scale = 1.0 / float(np.sqrt(HD))

    nc = bacc.Bacc("TRN2", target_bir_lowering=False, debug=False,
                   num_devices=n_cores)
    xt = nc.dram_tensor("xt", [DIM, NT], BF16, kind="ExternalInput").ap()
    maskd = nc.dram_tensor("maskc", [P, MW], BF16, kind="ExternalInput").ap()
    wq = nc.dram_tensor("wq", [DIM, DHC], BF16, kind="ExternalInput").ap()
    wk = nc.dram_tensor("wk", [DIM, DHC], BF16, kind="ExternalInput").ap()
    wv = nc.dram_tensor("wv", [DIM, DHC], BF16, kind="ExternalInput").ap()
    wo = nc.dram_tensor("wo", [DHC, DIM], BF16, kind="ExternalInput").ap()
    cc = nc.dram_tensor("cc", [HD, S], BF16, kind="ExternalInput").ap()
    ss = nc.dram_tensor("ss", [HD, S], BF16, kind="ExternalInput").ap()
    onesb = nc.dram_tensor("onesb", [P, 1], BF16, kind="ExternalInput").ap()
    out = nc.dram_tensor("out", [NT, DIM], BF16, kind="ExternalOutput").ap()

    with ExitStack() as ctx:
        tc = ctx.enter_context(tile.TileContext(nc))
        wpool = ctx.enter_context(tc.tile_pool(name="weights", bufs=1))
        xpool = ctx.enter_context(tc.tile_pool(name="xtp", bufs=xt_bufs))
        qrawp = ctx.enter_context(tc.tile_pool(name="qraw", bufs=qraw_bufs))
        rotp = ctx.enter_context(tc.tile_pool(name="rot", bufs=rot_bufs))
        qfp = ctx.enter_context(tc.tile_pool(name="qfp", bufs=qfq_bufs))
        kfp = ctx.enter_context(tc.tile_pool(name="kfp", bufs=kf_bufs))
        vnp = ctx.enter_context(tc.tile_pool(name="vn", bufs=vn_bufs))
        expp = ctx.enter_context(tc.tile_pool(name="expp", bufs=exp_bufs))
        eaccp = ctx.enter_context(tc.tile_pool(name="eacc", bufs=eacc_bufs))
        rsp = ctx.enter_context(tc.tile_pool(name="rs", bufs=rs_bufs))
        rsfp = ctx.enter_context(tc.tile_pool(name="rsf", bufs=rsf_bufs))
        otp = ctx.enter_context(tc.tile_pool(name="ot", bufs=ot_bufs))
        opp = ctx.enter_context(tc.tile_pool(name="op", bufs=op_bufs))
        psum = ctx.enter_context(tc.tile_pool(name="ps", bufs=psum_bufs,
                                              space="PSUM"))

        # ---- persistent constants ----
        wq_t = [wpool.tile([P, DHC], BF16, tag=f"wq{kc}", name=f"wq{kc}")
                for kc in range(KC)]
        wk_t = [wpool.tile([P, DHC], BF16, tag=f"wk{kc}", name=f"wk{kc}")
                for kc in range(KC)]
        wv_t = [wpool.tile([P, DHC], BF16, tag=f"wv{kc}", name=f"wv{kc}")
                for kc in range(KC)]
        for kc in range(KC):
            nc.gpsimd.dma_start(wq_t[kc][:], wq[kc * P:(kc + 1) * P, :])
            nc.gpsimd.dma_start(wk_t[kc][:], wk[kc * P:(kc + 1) * P, :])
            nc.gpsimd.dma_start(wv_t[kc][:], wv[kc * P:(kc + 1) * P, :])
        wo_t = [wpool.tile([P, DIM], BF16, tag=f"wo{h}", name=f"wo{h}")
                for h in range(HPC)]
        for h in range(HPC):
            nc.gpsimd.dma_start(wo_t[h][:], wo[h * HD:(h + 1) * HD, :])
        cc_t = wpool.tile([HD, S], BF16, tag="cc")
        ss_t = wpool.tile([HD, S], BF16, tag="ss")
        nc.gpsimd.dma_start(cc_t[:], cc[:, :])
        nc.gpsimd.dma_start(ss_t[:], ss[:, :])
        maskc = wpool.tile([P, MW], BF16, tag="maskc")
        nc.gpsimd.dma_start(maskc[:], maskd[:, :])
        ones_col = wpool.tile([P, 1], BF16, tag="ones_col")
        nc.gpsimd.dma_start(ones_col[:], onesb[:, :])

        def mask_j(j):
            off = (JD - 1 - j) * P
            return maskc[:, off:off + QT]

        # xt tile prefetch bookkeeping: xts[(b, t, kc)] -> tile
        xts = {}

        def emit_xt_dmas(b):
            tok0 = b * S
            for t in range(NQT):
                for kc in range(KC):
                    xtile = xpool.tile([P, QT], BF16, tag="xt", name="xt")
                    nc.sync.dma_start(
                        xtile[:],
                        xt[kc * P:(kc + 1) * P,
                           tok0 + t * QT:tok0 + (t + 1) * QT])
                    xts[(b, t, kc)] = xtile

        # pending output-projection units (closures), drained to keep PE busy
        pending = []
        fin_pending = []

        def drain(k=None):
            n = len(pending) if k is None else min(k, len(pending))
            for _ in range(n):
                pending.pop(0)()

        def drain_fin(k=1):
            n = len(fin_pending) if k is None else min(k, len(fin_pending))
            for _ in range(n):
                fin_pending.pop(0)()

        def emit_rope(ps_t, dest, tsl):
            """RoPE from psum ps_t -> bf16 dest ([:, tsl] of cc/ss)."""
            raw = qrawp.tile([P, QT], BF16, tag="qraw", name="qraw")
            nc.scalar.copy(raw[:], ps_t[:])
            rot = rotp.tile([P, QT], BF16, tag="rot", name="rot")
            nc.scalar.copy(rot[0:HD // 2, :], raw[HD // 2:HD, :])
            nc.scalar.copy(rot[HD // 2:HD, :], raw[0:HD // 2, :])
            nc.vector.tensor_mul(rot[:], rot[:], ss_t[:, tsl])
            nc.vector.tensor_mul(dest, raw[:], cc_t[:, tsl])
            nc.vector.tensor_add(dest, dest, rot[:])

        for rep in range(reps):
          for b in range(B):
            tok0 = b * S
            if rep == 0 and b == 0:
                emit_xt_dmas(0)
            # ---------- phase A: QKV projections + RoPE ----------
            qf = [[None] * NQT for _ in range(HPC)]
            kf = [kfp.tile([P, S], BF16, tag=f"kf{h}", name=f"kf{h}")
                  for h in range(HPC)]
            vn = [vnp.tile([P, DHC], BF16, tag="vn", name="vn")
                  for _ in range(SC)]
            for t in range(NQT):
                tsl = slice(t * QT, (t + 1) * QT)
                qps = [psum.tile([P, QT], F32, tag="ps", name="ps")
                       for _ in range(HPC)]
                kps = [psum.tile([P, QT], F32, tag="ps", name="ps")
                       for _ in range(HPC)]
                for kc in range(KC):
                    xtile = xts[(b, t, kc)]
                    st = dict(start=(kc == 0), stop=(kc == KC - 1))
                    for h in range(HPC):
                        hsl = slice(h * HD, (h + 1) * HD)
                        nc.tensor.matmul(qps[h][:], wq_t[kc][:, hsl],
                                         xtile[:], **st)
                        nc.tensor.matmul(kps[h][:], wk_t[kc][:, hsl],
                                         xtile[:], **st)
                    if t == 0 and kc in (1, 3):
                        drain_fin()  # prev batch qt=3 finalize stages
                # pending outproj from the previous batch's last qt
                if t == 0:
                    drain()
                # RoPE (ACT+DVE; overlaps the vnat matmuls below on PE)
                for h in range(HPC):
                    qf[h][t] = qfp.tile([P, QT], BF16, tag=f"qf{h}",
                                        name=f"qf{h}")
                    emit_rope(qps[h], qf[h][t][:], tsl)
                    emit_rope(kps[h], kf[h][:, tsl], tsl)
                # v in natural layout: vn[tok128, dv] = sum_kc xt_c.T @ wv_c
                for tcl in range(JD):
                    vp = psum.tile([P, DHC], F32, tag="ps", name="ps")
                    csl = slice(tcl * P, (tcl + 1) * P)
                    for kc in range(KC):
                        nc.tensor.matmul(vp[:], xts[(b, t, kc)][:, csl],
                                         wv_t[kc][:], start=(kc == 0),
                                         stop=(kc == KC - 1))
                    nc.scalar.copy(vn[t * JD + tcl][:], vp[:])

            # ---------- phase B: attention + pipelined outproj ----------
            if b + 1 < B:
                emit_xt_dmas(b + 1)
            elif rep + 1 < reps:
                emit_xt_dmas(0)
            for qt in range(NQT):
                n_kc = JD * (qt + 1)  # causal: key chunks 0..n_kc-1
                avs = [psum.tile([P, QT], F32, tag="ps", name="ps")
                       for _ in range(HPC)]
                eacc = [eaccp.tile([P, QT], BF16, tag="eacc", name="eacc")
                        for _ in range(HPC)]
                ess = [[None] * n_kc for _ in range(HPC)]

                def emit_sc(h, i, qt=qt, ess=ess, eacc=eacc):
                    j = i - JD * qt
                    # causal narrowing: for diagonal chunk j, query columns
                    # < j*P are fully masked -- skip them in sc/exp/av
                    q0 = j * P if 0 < j < JD else 0
                    sc = psum.tile([P, QT], F32, tag="ps", name="ps")
                    nc.tensor.matmul(sc[:, q0:], kf[h][:, i * P:(i + 1) * P],
                                     qf[h][qt][:, q0:], start=True, stop=True)
                    e = expp.tile([P, QT], BF16, tag="exp", name="exp")
                    nc.scalar.activation(e[:, q0:], sc[:, q0:], AF.Exp,
                                         scale=scale)
                    if 0 <= j < JD:
                        tri = maskc[:, (JD - 1) * P:JD * P]
                        nc.vector.tensor_mul(e[:, q0:q0 + P],
                                             e[:, q0:q0 + P], tri)
                    with nc.allow_low_precision(reason="bf16 denom accum"):
                        if i == 0:
                            nc.vector.tensor_copy(eacc[h][:], e[:])
                        else:
                            nc.vector.tensor_add(eacc[h][:, q0:],
                                                 eacc[h][:, q0:], e[:, q0:])
                    ess[h][i] = (e, q0)

                def emit_av(h, i, n_kc=n_kc, avs=avs, ess=ess):
                    e, q0 = ess[h][i]
                    hsl = slice(h * HD, (h + 1) * HD)
                    nc.tensor.matmul(avs[h][:, q0:], vn[i][:, hsl],
                                     e[:, q0:], start=(i == 0),
                                     stop=(i == n_kc - 1),
                                     skip_group_check=True)
                    ess[h][i] = None

                dstride = max(1, (n_kc - drain_lo) // 4)
                dpos = {drain_lo + k * dstride for k in range(4)}
                for i in range(n_kc):
                    for h in range(HPC):
                        emit_sc(h, i)
                    if i == 1 or i == 3:
                        drain_fin()
                    if i >= look:
                        for h in range(HPC):
                            emit_av(h, i - look)
                    if i in dpos:
                        drain(1)
                for i in range(max(0, n_kc - look), n_kc):
                    for h in range(HPC):
                        emit_av(h, i)

                # F1 (deferred): denominators, reciprocal, broadcast, norm
                ots = []

                def fin1(avs=avs, eacc=eacc, ots=ots):
                    for h in range(HPC):
                        smp = psum.tile([1, QT], F32, tag="ps", name="ps")
                        nc.tensor.matmul(smp[:], ones_col[:], eacc[h][:],
                                         start=True, stop=True)
                        rs = rsp.tile([1, QT], F32R, tag="rs", name="rs")
                        with nc.allow_low_precision(reason="f32r width"):
                            nc.vector.reciprocal(rs[:], smp[:])
                        rsf = rsfp.tile([P, QT], F32R, tag="rsf", name="rsf")
                        nc.gpsimd.partition_broadcast(rsf[:], rs[:])
                        ot = otp.tile([P, QT], BF16, tag="ot", name="ot")
                        nc.vector.tensor_mul(ot[:], avs[h][:], rsf[:])
                        ots.append(ot)

                fin_pending.append(fin1)

                # F2: output projection, deferred via pending units
                def make_op_unit(tcl, ots=ots, qt=qt, tok0=tok0):
                    def unit():
                        csl = slice(tcl * P, (tcl + 1) * P)
                        r0 = tok0 + qt * QT + tcl * P
                        for mdt in range(MDT):
                            ps_t = psum.tile([P, QT], F32, tag="ps", name="ps")
                            mdsl = slice(mdt * QT, (mdt + 1) * QT)
                            for h in range(HPC):
                                nc.tensor.matmul(ps_t[:], ots[h][:, csl],
                                                 wo_t[h][:, mdsl],
                                                 start=(h == 0),
                                                 stop=(h == HPC - 1))
                            o = opp.tile([P, QT], BF16, tag="op", name="op")
                            if (tcl * MDT + mdt) % 8 < 3:
                                nc.scalar.copy(o[:], ps_t[:])
                            else:
                                nc.vector.tensor_copy(o[:], ps_t[:])
                            if (tcl * MDT + mdt) % 2 == 0:
                                nc.gpsimd.dma_start(
                                    out[r0:r0 + P, mdsl], o[:])
                            else:
                                nc.sync.dma_start(
                                    out[r0:r0 + P, mdsl], o[:])
                    return unit

                for tcl in range(JD):
                    pending.append(make_op_unit(tcl))
                if rep == reps - 1 and b == B - 1 and qt == NQT - 1:
                    drain_fin(None)
                    drain()
    return nc


def prep_shared(x, cos, sin, QT=512, P=128):
    """Host-side layout prep (transpose/concat/cast only, no FLOPs on x)."""
    import ml_dtypes
    bf16 = ml_dtypes.bfloat16
    B, S, DIM = x.shape
    JD = QT // P
    MW = (JD - 1) * P + QT
    g = np.arange(MW)[None, :]
    p = np.arange(P)[:, None]
    return dict(
        xt=np.ascontiguousarray(x.reshape(B * S, DIM).T).astype(bf16),
        cc=np.ascontiguousarray(
            np.concatenate([cos.T, cos.T], axis=0)).astype(bf16),
        ss=np.ascontiguousarray(
            np.concatenate([-sin.T, sin.T], axis=0)).astype(bf16),
        maskc=(g - (JD - 1) * P - p >= 0).astype(bf16),
        onesb=np.ones((P, 1), dtype=bf16),
    )


def shard_weights(wq, wk, wv, wo, core, n_cores=8, head_dim=128):
    import ml_dtypes
    bf16 = ml_dtypes.bfloat16
    n_heads = wq.shape[1] // head_dim
    hpc = n_heads // n_cores
    dhc = hpc * head_dim
    c0, c1 = core * dhc, (core + 1) * dhc
    return dict(
        wq=np.ascontiguousarray(wq[:, c0:c1]).astype(bf16),
        wk=np.ascontiguousarray(wk[:, c0:c1]).astype(bf16),
        wv=np.ascontiguousarray(wv[:, c0:c1]).astype(bf16),
        wo=np.ascontiguousarray(wo[c0:c1, :]).astype(bf16),
    )


# ---------------------------------------------------------------------------
# Self-contained entry point: kernel(**inputs) -> np.ndarray
# ---------------------------------------------------------------------------
import jax
from jax.sharding import Mesh, PartitionSpec
from jax.experimental.shard_map import shard_map

import concourse.bass2jax as bass2jax

N_CORES = 8
_CACHE = {}


def _get_runner():
    if "runner" in _CACHE:
        return _CACHE["runner"]
    nc = build_nc()
    nc.compile()
    bass2jax.install_neuronx_cc_hook()
    partition_name = (nc.partition_id_tensor.name
                      if nc.partition_id_tensor else None)
    in_names, out_names, out_avals, zero_outs = [], [], [], []
    for alloc in nc.m.functions[0].allocations:
        if not isinstance(alloc, mybir.MemoryLocationSet):
            continue
        name = alloc.memorylocations[0].name
        if alloc.kind == "ExternalInput":
            if name != partition_name:
                in_names.append(name)
        elif alloc.kind == "ExternalOutput":
            shape = tuple(alloc.tensor_shape)
            dtype = mybir.dt.np(alloc.dtype)
            out_names.append(name)
            out_avals.append(jax.core.ShapedArray(shape, dtype))
            zero_outs.append(np.zeros(shape, dtype))
    all_in_names = in_names + out_names
    if partition_name is not None:
        all_in_names = all_in_names + [partition_name]

    def _body(*args):
        operands = list(args)
        if partition_name is not None:
            operands.append(bass2jax.partition_id_tensor())
        outs = bass2jax._bass_exec_p.bind(
            *operands,
            out_avals=tuple(out_avals),
            in_names=tuple(all_in_names),
            out_names=tuple(out_names),
            lowering_input_output_aliases=(),
            sim_require_finite=True,
            sim_require_nnan=True,
            nc=nc,
        )
        return tuple(outs)

    devices = jax.devices()[:N_CORES]
    mesh = Mesh(np.asarray(devices), ("core",))
    n_in = len(in_names) + len(out_names)
    sharded = jax.jit(
        shard_map(_body, mesh=mesh,
                  in_specs=(PartitionSpec("core"),) * n_in,
                  out_specs=(PartitionSpec("core"),) * len(out_names),
                  check_rep=False),
        keep_unused=True,
        donate_argnums=tuple(range(len(in_names), n_in)))
    sharding = jax.sharding.NamedSharding(mesh, PartitionSpec("core"))
    _CACHE["runner"] = (sharded, in_names, out_names, out_avals, zero_outs,
                        sharding)
    return _CACHE["runner"]


def _device_inputs(x, cos, sin, wq, wk, wv, wo):
    shared = prep_shared(np.asarray(x, dtype=np.float32),
                         np.asarray(cos, dtype=np.float32),
                         np.asarray(sin, dtype=np.float32))
    in_maps = []
    for c in range(N_CORES):
        m = dict(shared)
        m.update(shard_weights(np.asarray(wq, dtype=np.float32),
                               np.asarray(wk, dtype=np.float32),
                               np.asarray(wv, dtype=np.float32),
                               np.asarray(wo, dtype=np.float32), c,
                               n_cores=N_CORES))
        in_maps.append(m)
    sharded, in_names, out_names, out_avals, zero_outs, sharding = \
        _get_runner()
    concat_in = [np.concatenate([np.asarray(in_maps[c][n])
                                 for c in range(N_CORES)], axis=0)
                 for n in in_names]
    concat_zero = [np.zeros((N_CORES * z.shape[0], *z.shape[1:]), z.dtype)
                   for z in zero_outs]
    dev_in = [jax.device_put(a, sharding) for a in concat_in + concat_zero]
    for a in dev_in:
        a.block_until_ready()
    return dev_in


def _gather(outs, B, S, DIM):
    full = np.asarray(outs[0]).reshape(N_CORES, B * S, DIM)
    return full.astype(np.float32).sum(axis=0).reshape(B, S, DIM)


def kernel(x, cos, sin, wq, wk, wv, wo):
    """Full inputs in, full output out; work sharded over 8 NeuronCores."""
    B, S, DIM = x.shape
    dev_in = _device_inputs(x, cos, sin, wq, wk, wv, wo)
    sharded = _get_runner()[0]
    outs = sharded(*dev_in)
    jax.block_until_ready(outs)
    return _gather(outs, B, S, DIM)


def measure_hw_time(x, cos, sin, wq, wk, wv, wo, k_lo=5, k_hi=105, trials=3):
    """Marginal per-call time of pipelined executions (min slope).

    Output buffers are donated, so each call's returned output feeds the
    next call's out-init operand (in-place aliasing on device).
    """
    import time as _time
    dev_in = _device_inputs(x, cos, sin, wq, wk, wv, wo)
    sharded, in_names = _get_runner()[:2]
    n_pure = len(in_names)
    pure_in = dev_in[:n_pure]
    state = list(dev_in[n_pure:])
    rs = sharded(*pure_in, *state)
    jax.block_until_ready(rs)
    state = list(rs)

    def timed(k, state):
        t0 = _time.time()
        for _ in range(k):
            state = list(sharded(*pure_in, *state))
        jax.block_until_ready(state)
        return _time.time() - t0, state

    slopes = []
    for _ in range(trials):
        t_lo, state = timed(k_lo, state)
        t_hi, state = timed(k_hi, state)
        slopes.append((t_hi - t_lo) / (k_hi - k_lo))
    return min(slopes)
